# revision 58
# baseline (speedup 1.0000x reference)
"""GCN decoder kernel for Trainium2 (8 NeuronCores, data-parallel batch).

Per batch element b (E=1024 nodes, D=H=768), all matmuls fp8 DoubleRow:
  S    = X X^T + ones(x)u8[f]   (u8 = -240*(1-m) folded into PSUM by an extra
                                 fp8-DR "mask matmul" so sigmoid(masked) == 0)
  Ahat = sigmoid(S)             (ACT evicts PSUM -> fp8 pair tiles directly,
                                 accum_out captures the masked row sums)
  deg  = (rowsum+1)*m -> dinv = (max(deg,1e-6))^-0.5 ; dm = dinv*m
  X'   = XF * dm[f]             (folds D^-1/2[f] + row mask into stage-2 lhsT)
  outT = X'^T @ Ahat, * dinv[e] on eviction (DROW broadcast via DRAM bounce)
  HfT  = relu(Wg^T @ outT + bg)    (DVE tensor_scalar add+max eviction)
  PT   = Wp^T @ HfT + bp           (Pool tensor_scalar bias eviction)
  S2   = PT^T @ PT + ones(x)u8[f]  (mask matmul again)
  out  = sigmoid(S2 + u[e])        (row mask via ACT bias; ACT evicts fp32,
                                    store from the ACT queue)

Sharding: batch 16 -> 2 per core across 8 cores; weights replicated.
"""

import sys

if "/opt/trn_rl_repo" not in sys.path:
    sys.path.insert(0, "/opt/trn_rl_repo")

from contextlib import ExitStack

import numpy as np
import ml_dtypes

import concourse.bass as bass
import concourse.tile as tile
from concourse import bacc, mybir
from concourse.bass_utils import run_bass_kernel_spmd

B, E, D, H = 16, 1024, 768, 768
NCORES = 8
BL = B // NCORES          # batch elements per core
ET = E // 128             # 8 e/f tiles
KD = D // 128             # 6 d/h tiles
FC = E // 512             # 2 moving chunks of 512
KP = D // 256             # 3 fp8 DoubleRow pair-tiles over d/h
KF = E // 256             # 4 fp8 DoubleRow pair-tiles over f

FP32 = mybir.dt.float32
BF16 = mybir.dt.bfloat16
FP8 = mybir.dt.float8e4
I32 = mybir.dt.int32
AL = mybir.AluOpType
AF = mybir.ActivationFunctionType
DR = mybir.MatmulPerfMode.DoubleRow

UVAL = -240.0             # fp8-exact; |S| off-diag < 190, S2 < 1

_cached_nc = {}


def _build(loops=1):
    if loops in _cached_nc:
        return _cached_nc[loops]

    nc = bacc.Bacc("TRN2", target_bir_lowering=False, debug=False)

    xp_d = nc.dram_tensor("XP", [BL, 128, KP, 2, E], FP8, kind="ExternalInput")
    xf_d = nc.dram_tensor("XF", [BL, 128, KF, 2, D], FP8, kind="ExternalInput")
    wg_d = nc.dram_tensor("WG", [128, KP, 2, H], FP8, kind="ExternalInput")
    wp_d = nc.dram_tensor("WP", [128, KP, 2, H], FP8, kind="ExternalInput")
    bgbp_d = nc.dram_tensor("BGBP", [128, 2, KD], FP32, kind="ExternalInput")
    uf_d = nc.dram_tensor("UF", [BL, 128, 2, E], FP8, kind="ExternalInput")
    one_d = nc.dram_tensor("ONE8", [128, 2, 128], FP8, kind="ExternalInput")
    mcub_d = nc.dram_tensor("MCUB", [BL, 128, 2, ET], FP32, kind="ExternalInput")
    eye_d = nc.dram_tensor("EYE", [128, 128], BF16, kind="ExternalInput")
    out_d = nc.dram_tensor("OUT", [BL, E, E], BF16, kind="ExternalOutput")

    with tile.TileContext(nc) as tc, ExitStack() as ctx:
        ep = ctx.enter_context
        wpool = ep(tc.tile_pool(name="wpool", bufs=1))
        xtp = ep(tc.tile_pool(name="xt", bufs=2))
        xfp = ep(tc.tile_pool(name="xf", bufs=2))
        ahp = ep(tc.tile_pool(name="ahat", bufs=2))
        rows = ep(tc.tile_pool(name="rows", bufs=2))
        stat = ep(tc.tile_pool(name="stat", bufs=2))
        oftp = ep(tc.tile_pool(name="oft", bufs=2))
        hftp = ep(tc.tile_pool(name="hft", bufs=2))
        ptp = ep(tc.tile_pool(name="ptp", bufs=2))
        rowtmp = ep(tc.tile_pool(name="rowtmp", bufs=2))
        drowp = ep(tc.tile_pool(name="drowp", bufs=2))
        ostp = ep(tc.tile_pool(name="ost", bufs=8))
        pspool = ep(tc.tile_pool(name="psum", bufs=4, space="PSUM"))
        dscr = ep(tc.tile_pool(name="dscr", bufs=2, space="DRAM"))

        XT = [None] * BL
        XF8 = [None] * BL
        AH = [[None] * KF for _ in range(BL)]
        UF = [None] * BL
        MCOL = [None] * BL
        UB = [None] * BL
        DROW = [None] * BL
        DM = [None] * BL
        RS = [None] * BL
        OFT = [[None] * KP for _ in range(BL)]
        HFT = [[None] * KP for _ in range(BL)]
        PT = [[None] * KP for _ in range(BL)]

        one8 = wpool.tile([128, 2, 128], FP8, tag="one8")
        eye = wpool.tile([128, 128], BF16, tag="eye")

        def load_inputs(b):
            # per-k first halves land fast (PE start), later chunks larger
            t = xtp.tile([128, KP, 2, E], FP8, tag="xp")
            XT[b] = t
            nc.sync.dma_start(t[:, :, :, 0:512], xp_d[b, :, :, :, 0:512])
            if b == 0:
                nc.sync.dma_start(one8[:], one_d[:])
            uf = rows.tile([128, 2, E], FP8, tag="uf")
            nc.sync.dma_start(uf[:], uf_d[b, :, :, :])
            UF[b] = uf
            nc.sync.dma_start(t[:, :, :, 512:E], xp_d[b, :, :, :, 512:E])
            if b == 0:
                nc.sync.dma_start(eye[:], eye_d[:])
            mcub = rows.tile([128, 2, ET], FP32, tag="mcub")
            nc.sync.dma_start(mcub[:], mcub_d[b, :, :, :])
            MCOL[b] = mcub[:, 0, :]
            UB[b] = mcub[:, 1, :]

        def load_stage2_inputs(b):
            # gpsimd SWDGE: one descriptor, fits in Pool's idle head before
            # the first row-sum STT
            t = xfp.tile([128, KF, 2, D], FP8, tag="xf")
            nc.gpsimd.dma_start(t[:], xf_d[b, :, :, :, :])
            XF8[b] = t

        def stage1(b):
            psp = pspool
            rs = stat.tile([128, ET], FP32, tag="rs")
            for et in range(ET):
                if et % 2 == 0:
                    ah = ahp.tile([128, 2, E], FP8, tag=f"ah{et // 2}")
                    AH[b][et // 2] = ah
                else:
                    ah = AH[b][et // 2]
                ps = psp.tile([128, E], FP32)
                # fc outer: the fc=0 groups touch only first-half XP columns,
                # so the PE can start before the second-half DMAs land.
                # Last matmul of each group adds ones[e]*u8[f] into PSUM so
                # sigmoid(masked col) == 0.
                for fc in range(FC):
                    for k in range(KP):
                        nc.tensor.matmul(
                            ps[:, fc * 512:(fc + 1) * 512],
                            XT[b][:, k, :, et * 128:(et + 1) * 128],
                            XT[b][:, k, :, fc * 512:(fc + 1) * 512],
                            start=(k == 0),
                            stop=False,
                            perf_mode=DR,
                        )
                    nc.tensor.matmul(
                        ps[:, fc * 512:(fc + 1) * 512],
                        one8[:],
                        UF[b][:, :, fc * 512:(fc + 1) * 512],
                        start=False,
                        stop=True,
                        perf_mode=DR,
                    )
                # ACT evicts masked sigmoid straight to fp8; accum_out
                # captures the masked row sums for the degree chain
                nc.scalar.activation(
                    ah[:, et % 2, :], ps[:], AF.Sigmoid,
                    accum_out=rs[:, et:et + 1],
                )
                # self-loop on the diagonal 128-block (Pool, SBUF-only)
                nc.gpsimd.tensor_add(
                    ah[:, et % 2, et * 128:(et + 1) * 128],
                    ah[:, et % 2, et * 128:(et + 1) * 128],
                    eye[:],
                )

            RS[b] = rs

        def chain1(b):
            # degree -> dinv chain ([128, 8] per-partition layout), all DVE.
            # dinv = deg^-1/2 via quake-style rsqrt (no ACT, so the whole
            # kernel needs one act-table load). For b0 this is emitted right
            # after stage1(b0) (it gates stage2(b0)); for b1 it is deferred
            # until after stage4(b0), where DVE sits idle during stage5(b0),
            # so it never head-of-line blocks DVE's stage2-4 evictions.
            rs = RS[b]
            deg = stat.tile([128, ET], FP32, tag="deg")
            nc.vector.scalar_tensor_tensor(
                out=deg[:], in0=rs[:], scalar=1.0, in1=MCOL[b],
                op0=AL.add, op1=AL.mult,
            )
            nc.vector.tensor_scalar_max(deg[:], deg[:], 1e-6)
            yi = stat.tile([128, ET], I32, tag="yi")
            nc.vector.tensor_scalar(
                out=yi[:], in0=deg[:].bitcast(I32), scalar1=1, scalar2=-1,
                op0=AL.logical_shift_right, op1=AL.bitwise_xor,
            )
            nc.vector.tensor_scalar(
                out=yi[:], in0=yi[:], scalar1=0x5F3759DF + 1, scalar2=None,
                op0=AL.add,
            )
            dinv = yi[:].bitcast(FP32)
            sqt = stat.tile([128, ET], FP32, tag="sqt")
            for _ in range(2):
                nc.vector.tensor_mul(sqt[:], dinv, dinv)
                nc.vector.tensor_mul(sqt[:], deg[:], sqt[:])
                nc.vector.tensor_scalar(
                    out=sqt[:], in0=sqt[:], scalar1=-0.5, scalar2=1.5,
                    op0=AL.mult, op1=AL.add,
                )
                nc.vector.tensor_mul(dinv, dinv, sqt[:])
            dm = stat.tile([128, ET], FP32, tag="dm")
            nc.vector.tensor_mul(dm[:], dinv, MCOL[b])
            DM[b] = dm

            # dinv broadcast row via DRAM bounce + partition_broadcast
            dsc = dscr.tile([128, ET], FP32, tag="dsc")
            nc.sync.dma_start(dsc[:], dinv)
            drow1 = rowtmp.tile([1, E], FP32, tag="drow1")
            nc.sync.dma_start(drow1[0:1, :], dsc[:, :].rearrange("p t -> t p"))
            drow = drowp.tile([128, E], FP32, tag="drow")
            nc.gpsimd.partition_broadcast(drow[:], drow1[0:1, :])
            DROW[b] = drow[:]

            # scale AH rows by dm[e] in place (equivalent to scaling X by
            # dm on the stage-2 lhsT side, but half the elements)
            for k in range(KF):
                for j in range(2):
                    nc.vector.tensor_scalar_mul(
                        AH[b][k][:, j, :], AH[b][k][:, j, :],
                        dm[:, 2 * k + j:2 * k + j + 1],
                    )

        def stage2(b):
            psp = pspool
            for dt in range(KD):
                if dt % 2 == 0:
                    oft = oftp.tile([128, 2, E], FP8, tag=f"oft{dt // 2}")
                    OFT[b][dt // 2] = oft
                ps = psp.tile([128, E], FP32)
                for k in range(KF):
                    for ec in range(FC):
                        nc.tensor.matmul(
                            ps[:, ec * 512:(ec + 1) * 512],
                            XF8[b][:, k, :, dt * 128:(dt + 1) * 128],
                            AH[b][k][:, :, ec * 512:(ec + 1) * 512],
                            start=(k == 0),
                            stop=(k == KF - 1),
                            perf_mode=DR,
                        )
                # DVE only: Pool can't read PSUM, ACT can't scale per-column
                nc.vector.tensor_mul(
                    OFT[b][dt // 2][:, dt % 2, :], ps[:], DROW[b]
                )

        def stage3(b):
            psp = pspool
            for ht in range(KD):
                if ht % 2 == 0:
                    hf = hftp.tile([128, 2, E], FP8, tag=f"hft{ht // 2}")
                    HFT[b][ht // 2] = hf
                ps = psp.tile([128, E], FP32)
                for k in range(KP):
                    for ec in range(FC):
                        nc.tensor.matmul(
                            ps[:, ec * 512:(ec + 1) * 512],
                            wg[:, k, :, ht * 128:(ht + 1) * 128],
                            OFT[b][k][:, :, ec * 512:(ec + 1) * 512],
                            start=(k == 0),
                            stop=(k == KP - 1),
                            perf_mode=DR,
                        )
                # relu(x + bg): rotate ACT/DVE (relu is in every ACT table
                # set, so no table load; Pool can't read PSUM)
                if ht % 2 == 0:
                    nc.scalar.activation(
                        HFT[b][ht // 2][:, ht % 2, :], ps[:], AF.Relu,
                        bias=bgbp[:, 0, ht:ht + 1],
                    )
                else:
                    nc.vector.tensor_scalar(
                        out=HFT[b][ht // 2][:, ht % 2, :], in0=ps[:],
                        scalar1=bgbp[:, 0, ht:ht + 1], scalar2=0.0,
                        op0=AL.add, op1=AL.max,
                    )

        def stage4(b):
            psp = pspool
            for ht in range(KD):
                if ht % 2 == 0:
                    pt = ptp.tile([128, 2, E], FP8, tag=f"pt{ht // 2}")
                    PT[b][ht // 2] = pt
                ps = psp.tile([128, E], FP32)
                for k in range(KP):
                    for ec in range(FC):
                        nc.tensor.matmul(
                            ps[:, ec * 512:(ec + 1) * 512],
                            wp[:, k, :, ht * 128:(ht + 1) * 128],
                            HFT[b][k][:, :, ec * 512:(ec + 1) * 512],
                            start=(k == 0),
                            stop=(k == KP - 1),
                            perf_mode=DR,
                        )
                # x + bp: rotate ACT/DVE; DVE takes the even ht so its last
                # eviction lands early and stage5's Ldweights aren't starved
                if ht % 2 == 1:
                    nc.scalar.activation(
                        PT[b][ht // 2][:, ht % 2, :], ps[:], AF.Identity,
                        bias=bgbp[:, 1, ht:ht + 1],
                    )
                else:
                    nc.vector.tensor_scalar_add(
                        PT[b][ht // 2][:, ht % 2, :], ps[:], bgbp[:, 1, ht:ht + 1]
                    )

        def stage5(b):
            psp = pspool
            for et in range(ET):
                ost = ostp.tile([128, E], BF16)
                ps = psp.tile([128, E], FP32)
                for k in range(KP):
                    for fc in range(FC):
                        nc.tensor.matmul(
                            ps[:, fc * 512:(fc + 1) * 512],
                            PT[b][k][:, :, et * 128:(et + 1) * 128],
                            PT[b][k][:, :, fc * 512:(fc + 1) * 512],
                            start=(k == 0),
                            stop=False,
                            perf_mode=DR,
                        )
                for fc in range(FC):
                    nc.tensor.matmul(
                        ps[:, fc * 512:(fc + 1) * 512],
                        one8[:],
                        UF[b][:, :, fc * 512:(fc + 1) * 512],
                        start=False,
                        stop=True,
                        perf_mode=DR,
                    )
                nc.scalar.activation(
                    ost[:], ps[:], AF.Sigmoid, bias=UB[b][:, et:et + 1],
                )
                # alternate DMA queues so OUT stores stream out in parallel
                deng = nc.scalar if et % 2 == 0 else nc.sync
                deng.dma_start(
                    out_d[b, et * 128:(et + 1) * 128, :], ost[:]
                )

        for b in range(BL):
            load_inputs(b)

        # ---- weights / constants (first use is stage 3) ----
        wg = wpool.tile([128, KP, 2, H], FP8, tag="wg")
        nc.sync.dma_start(wg[:], wg_d[:])
        wp = wpool.tile([128, KP, 2, H], FP8, tag="wp")
        nc.sync.dma_start(wp[:], wp_d[:])
        bgbp = wpool.tile([128, 2, KD], FP32, tag="bgbp")
        nc.sync.dma_start(bgbp[:], bgbp_d[:])

        for _ in range(loops):
            load_stage2_inputs(0)
            stage1(0)
            chain1(0)
            load_stage2_inputs(1)
            stage1(1)
            # chain stages per batch element: b0's OUT stores overlap b1's
            # compute; b1's degree/scale chain slots into DVE's idle window
            # during stage5(b0)
            stage2(0)
            stage3(0)
            stage4(0)
            chain1(1)
            stage5(0)
            stage2(1)
            stage3(1)
            stage4(1)
            stage5(1)

    nc.compile()
    _cached_nc[loops] = nc
    return nc


def make_in_maps(X, mask, W_gcn, b_gcn, W_proj, b_proj):
    bf = ml_dtypes.bfloat16
    f8 = mybir.dt.np(FP8)
    X = np.ascontiguousarray(np.asarray(X, dtype=np.float32))
    m = np.asarray(mask).astype(np.float32)
    u = (-2000.0 * (1.0 - m)).astype(np.float32)          # (B, E)
    u8 = (UVAL * (1.0 - m)).astype(np.float32)            # (B, E) fp8-exact
    wgT = np.asarray(W_gcn, np.float32).T
    wpT = np.asarray(W_proj, np.float32).T
    # [128, KP, 2, H]: partition-major single-DMA layout
    wg = np.ascontiguousarray(
        wgT.reshape(KP, 2, 128, H).transpose(2, 0, 1, 3)).astype(f8)
    wp = np.ascontiguousarray(
        wpT.reshape(KP, 2, 128, H).transpose(2, 0, 1, 3)).astype(f8)
    bg = np.asarray(b_gcn, np.float32).reshape(KD, 128).T
    bp = np.asarray(b_proj, np.float32).reshape(KD, 128).T
    bgbp = np.ascontiguousarray(np.stack([bg, bp], axis=1))  # [128, 2, KD]
    eye = np.eye(128, dtype=bf)
    one8 = np.zeros((128, 2, 128), dtype=f8)
    one8[0, 0, :] = 1.0
    in_maps = []
    for c in range(NCORES):
        sl = slice(c * BL, (c + 1) * BL)
        Xc = X[sl]
        mc = m[sl]
        uc = u[sl]
        uf = np.zeros((BL, 128, 2, E), dtype=f8)
        uf[:, 0, 0, :] = u8[sl]
        mcol = mc.reshape(BL, ET, 128).transpose(0, 2, 1)
        ubb = uc.reshape(BL, ET, 128).transpose(0, 2, 1)
        mcub = np.ascontiguousarray(
            np.stack([mcol, ubb], axis=2))                   # [BL,128,2,ET]
        in_maps.append({
            "XP": np.ascontiguousarray(
                Xc.transpose(0, 2, 1).reshape(BL, KP, 2, 128, E)
                .transpose(0, 3, 1, 2, 4)
            ).astype(f8),
            "XF": np.ascontiguousarray(
                Xc.reshape(BL, KF, 2, 128, D).transpose(0, 3, 1, 2, 4)
            ).astype(f8),
            "WG": wg,
            "WP": wp,
            "BGBP": bgbp,
            "UF": uf,
            "ONE8": one8,
            "MCUB": mcub,
            "EYE": eye,
        })
    return in_maps


def kernel(X, mask, W_gcn, b_gcn, W_proj, b_proj):
    nc = _build()
    in_maps = make_in_maps(X, mask, W_gcn, b_gcn, W_proj, b_proj)
    res = run_bass_kernel_spmd(nc, in_maps, list(range(NCORES)))
    out = np.concatenate([r["OUT"] for r in res.results], axis=0)
    return np.ascontiguousarray(out.astype(np.float32))
